# revision 23
# baseline (speedup 1.0000x reference)
"""Single-head causal attention (B=8, S=2048, D=1024, H=64).

Data-parallel over batch: each of the 8 NeuronCores computes one batch
element's full attention head.  Per core:

  qT/kT = (Wq|Wk)^T @ idx^T          -> [128, S]  (rows 0-63 qT, 64-127 kT)
  vT    = Wv^T @ idx^T               -> col-packed pairs of q-blocks
  v     = transpose(vT)              -> [S/128 x 128, 64] + ones column
  sT[k,q] = kT_tile^T @ qT           -> row-packed pairs (k on partitions)
  p = exp(sT / sqrt(D)) * causal     (no max subtraction: |s| <= ~2)
  oT[65, q] += v_aug[k]^T @ p        -> rows 0-63 out, row 64 = sum(exp) = Z

The kernel stores oT = [65, S] f32 (unnormalized + Z row); the HOST does
out[q, h] = oT[h, q] / oT[64, q] -- no on-chip transpose or divide.

Schedule: the framework preamble releases the engines at ~7us; from
there everything is input-DMA-gated (4.3MB bf16 at ~360-500 GB/s), so
idx blocks stream on sync+scalar HWDGE rings in block order while
weights ride the gpsimd SWDGE ring.  Attention is a qb-major pipeline:
scores for pair m+1/m+2 overlap exp of pair m on the ACT engine
(exp ~20us serial is the co-pole with the PE at ~23us), AV matmuls
trail exp, projections fill PE slack as their block lands.
"""

import sys

for _p in ("/opt/trn_rl_repo",):
    if _p not in sys.path:
        sys.path.insert(0, _p)

import numpy as np
import ml_dtypes

import concourse.bacc as bacc
import concourse.bass as bass
import concourse.mybir as mybir
from concourse import masks, tile
from concourse.bass_utils import run_bass_kernel_spmd

B, S, D, H = 8, 2048, 1024, 64
P = 128
QB = 512            # q-block width (one PSUM bank of f32)
NB = S // QB        # 4 q-blocks
KT = S // P         # 16 k-tiles
DT = D // P         # 8 d-tiles
SCALE = float(D) ** -0.5  # 1/32, exact in bf16/f32

BF16 = mybir.dt.bfloat16
F32 = mybir.dt.float32
AF = mybir.ActivationFunctionType

TRACE = False
LAST_RESULT = None


def enable_trace():
    """Register the NTFF profile hook that the agent image's antenv lacks,
    and neuter the artifact upload (no bucket in this container)."""
    global TRACE
    import types

    import antenv
    import concourse.bass_utils as bu

    if "antenv.axon_hooks" not in sys.modules:
        mod = types.ModuleType("antenv.axon_hooks")
        mod._hook = None
        mod.set_axon_ntff_profile_hook = lambda h: setattr(mod, "_hook", h)
        mod.get_axon_ntff_profile_hook = lambda: mod._hook
        sys.modules["antenv.axon_hooks"] = mod
        antenv.axon_hooks = mod
    from trn_agent_boot.trn_boot import _ntff_profile_via_ctypes

    sys.modules["antenv.axon_hooks"].set_axon_ntff_profile_hook(
        _ntff_profile_via_ctypes("/opt/axon/libaxon_pjrt.so")
    )
    bu.upload_artifacts = lambda tmpdir: tmpdir
    TRACE = True


def build_bass():
    nc = bacc.Bacc("TRN2", target_bir_lowering=False, debug=False, num_devices=B)
    # block-major swizzle: idxT_d[b, p, t*QB + s] = idx[b*QB + s, t*P + p]
    idxT_d = nc.declare_dram_parameter("idxT", [NB, P, DT * QB], BF16, isOutput=False)
    # host pre-swizzled: wqk_d[p, t*2H + m] = concat(Wq,Wk)[t*P + p, m]
    wqk_d = nc.declare_dram_parameter("wqk", [P, DT * 2 * H], BF16, isOutput=False)
    wv_d = nc.declare_dram_parameter("wv", [P, DT * H], BF16, isOutput=False)
    # oT[h, q]: rows 0-63 unnormalized out, row 64 = Z (host divides)
    out_d = nc.declare_dram_parameter("out", [H + 1, S], F32, isOutput=True)

    with tile.TileContext(nc) as tc:
        with (
            tc.tile_pool(name="consts", bufs=1) as consts,
            tc.tile_pool(name="data", bufs=1) as data,
            tc.tile_pool(name="pp", bufs=8) as pp,
            tc.tile_pool(name="ps_mm", bufs=3, space="PSUM") as ps_mm,
            tc.tile_pool(name="ps_pj", bufs=1, space="PSUM") as ps_pj,
            tc.tile_pool(name="ps_o", bufs=1, space="PSUM") as ps_o,
        ):
            # ---------------- input DMAs first ----------------
            # idx blocks in need-order: 0a/0b/1 on the sync HWDGE ring,
            # 2/3 on the scalar HWDGE ring, weights on the gpsimd SWDGE
            idxT_sb = data.tile([P, NB, DT, QB], BF16)

            def load_idx(eng, b, splits=1):
                src = idxT_d[b].rearrange("p (t s) -> p t s", t=DT)
                step = DT // splits
                for u in range(splits):
                    ts = slice(u * step, (u + 1) * step)
                    eng.dma_start(idxT_sb[:, b, ts, :], src[:, ts, :])

            # weights ride the sync HWDGE ring (fast, lands ~9.5us); all idx
            # blocks stream in block order on the scalar HWDGE ring, which
            # alone sustains ~360 B/ns; the sync ring then carries the
            # output stores.  The dummy exp (ACT table load, ~1.3us queue
            # time) slots between idx0's and idx1's issues so the table is
            # resident well before the first real exp without delaying idx0.
            src0 = idxT_d[0].rearrange("p (t s) -> p t s", t=DT)
            nc.sync.dma_start(idxT_sb[:, 0, 0:4, :], src0[:, 0:4, :])
            wqk_sb = consts.tile([P, DT, 2 * H], BF16)
            nc.scalar.dma_start(wqk_sb[:], wqk_d.rearrange("p (t m) -> p t m", t=DT))
            nc.scalar.dma_start(idxT_sb[:, 0, 4:8, :], src0[:, 4:8, :])
            wv_sb = consts.tile([P, DT, H], BF16)
            nc.sync.dma_start(wv_sb[:], wv_d.rearrange("p (t m) -> p t m", t=DT))
            dum = consts.tile([1, 16], F32)
            nc.scalar.activation(dum[:, 0:8], dum[:, 8:16], AF.Exp)
            load_idx(nc.scalar, 1)
            load_idx(nc.scalar, 2)
            load_idx(nc.scalar, 3, splits=2)

            # ---------------- PE warmup ----------------
            junk = consts.tile([P, QB], BF16)
            nc.vector.memset(junk[:], 0.0)
            wps = ps_pj.tile([P, QB], F32, tag="pj")
            for _ in range(10):
                nc.tensor.matmul(wps[:], junk[:, 0:P], junk[:], start=True, stop=True)

            # ---------------- constants ----------------
            ident = consts.tile([P, P], BF16)
            masks.make_identity(nc, ident[:])
            # mask[k, q] = 1.0 where q >= k (upper triangular incl diagonal)
            mask_sb = consts.tile([P, P], BF16)
            masks.make_upper_triangular(nc, mask_sb[:], val=1.0, diag=True)
            # half-swap permutation: ident_sw[p, i] = 1 iff |p - i| == 64;
            # matmul with it as lhsT moves partitions 64-127 <-> 0-63
            ident_sw = consts.tile([P, P], BF16)
            nc.gpsimd.memset(ident_sw[:], 0.0)
            for sw_base in (H, -H):
                nc.gpsimd.affine_select(
                    out=ident_sw[:],
                    in_=ident_sw[:],
                    compare_op=mybir.AluOpType.not_equal,
                    fill=1.0,
                    base=sw_base,
                    pattern=[[-1, P]],
                    channel_multiplier=1,
                )
            # identity on partitions 64-127 (for transposing hi-half vT)
            identh = consts.tile([P, H], BF16)
            nc.gpsimd.memset(identh[:], 0.0)
            nc.gpsimd.affine_select(
                out=identh[:],
                in_=identh[:],
                compare_op=mybir.AluOpType.not_equal,
                fill=1.0,
                base=-H,
                pattern=[[-1, H]],
                channel_multiplier=1,
            )

            # ---------------- working tiles ----------------
            qkT_sb = data.tile([P, S], BF16)   # rows 0-63 qT, rows 64-127 kT
            qkT2_sb = data.tile([P, S], BF16)  # swapped: rows 0-63 kT, 64-127 qT
            vT_sb = data.tile([P, S // 2], BF16)  # even blocks rows 0-63, odd 64-127
            v_sb = data.tile([P, KT, H + 1], BF16)  # [k, 64 v | 1.0]
            out_sb = data.tile([H + 1, S], F32)

            # ones column of v_aug (flash-style Z accumulator row)
            nc.vector.memset(v_sb[:, :, H : H + 1].rearrange("p t o -> p (t o)"), 1.0)

            def proj_qk(b, chunks=1):
                cols = slice(b * QB, (b + 1) * QB)
                ps = ps_pj.tile([P, QB], F32, tag="pj", name=f"pqk{b}")
                for t in range(DT):
                    nc.tensor.matmul(
                        ps[:],
                        wqk_sb[:, t, :],
                        idxT_sb[:, b, t, :],
                        start=(t == 0),
                        stop=(t == DT - 1),
                    )
                nc.vector.tensor_copy(qkT_sb[:, cols], ps[:])
                # scores row-packing needs kT on partitions 0-63 (pair slot A)
                # and qT on partitions 64-127 (pair slot B): swap the halves
                # on the PE via the permutation identity (no DMA latency)
                ps2 = ps_pj.tile([P, QB], F32, tag="pj", name=f"sw{b}")
                nc.tensor.matmul(
                    ps2[:], ident_sw[:], qkT_sb[:, cols], start=True, stop=True
                )
                nc.vector.tensor_copy(qkT2_sb[:, cols], ps2[:])

            def proj_v_pair(b):
                # col-packed pair: vT(b) -> psum parts 0-63, vT(b+1) -> 64-127
                hcols = slice(b * QB // 2, (b + 2) * QB // 2)
                ps = ps_pj.tile([P, QB], F32, tag="pj", name=f"pv{b}")
                for t in range(DT):
                    nc.tensor.matmul(
                        ps[0:H, :],
                        wv_sb[:, t, :],
                        idxT_sb[:, b, t, :],
                        start=(t == 0),
                        stop=(t == DT - 1),
                    )
                    nc.tensor.matmul(
                        ps[H:P, :],
                        wv_sb[:, t, :],
                        idxT_sb[:, b + 1, t, :],
                        start=(t == 0),
                        stop=(t == DT - 1),
                        tile_position=(0, H),
                    )
                nc.vector.tensor_copy(vT_sb[:, hcols], ps[:])

            def transp_v(b, g):
                # transpose 4 k-tiles of vT (one PSUM bank, single DVE evac)
                j0 = 4 * b + g
                pst = ps_pj.tile([P, 4, H], BF16, tag="pj", name=f"vt_{b}_{g}")
                for u in range(4):
                    j = j0 + u
                    jj = (j - 4 * b) % 4 + 2 * b  # column tile within vT_sb
                    hi = j >= 4 * b + 4
                    src = (
                        vT_sb[H:P, jj * P : (jj + 1) * P]
                        if hi
                        else vT_sb[0:H, jj * P : (jj + 1) * P]
                    )
                    nc.tensor.matmul(
                        pst[:, u, :],
                        src,
                        identh[H:P, :] if hi else ident[:H, :H],
                        is_transpose=True,
                        start=(u == 0),
                        stop=(u == 3),
                        skip_group_check=True,
                    )
                nc.vector.tensor_copy(v_sb[:, j0 : j0 + 4, 0:H], pst[:])

            # --------- attention: S-pair -> exp -> masked AV pair ---------
            pos = {}
            ptiles = {}

            def s_pair(qb, m):
                """Scores for k-tiles (2m, 2m+1) x q-block qb, then exp.
                B half is packed at column base QB (not QB+offb) so the
                single exp activation covers only valid columns."""
                ia, ib = 2 * m, 2 * m + 1
                offa = max(0, P * ia - QB * qb)
                offb = max(0, P * ib - QB * qb)
                ps = ps_mm.tile([P, 2 * QB], F32, tag="mm", name=f"s{qb}_{m}")
                nc.tensor.matmul(
                    ps[:, offa:QB],
                    qkT2_sb[0:H, ia * P : (ia + 1) * P],
                    qkT_sb[0:H, qb * QB + offa : (qb + 1) * QB],
                    start=True,
                    stop=True,
                )
                nc.tensor.matmul(
                    ps[:, QB : 2 * QB - offb],
                    qkT_sb[H:P, ib * P : (ib + 1) * P],
                    qkT2_sb[H:P, qb * QB + offb : (qb + 1) * QB],
                    start=True,
                    stop=True,
                )
                p_sb = pp.tile([P, 2 * QB], BF16, tag="p", name=f"p{qb}_{m}")
                nc.scalar.activation(
                    p_sb[:, offa : 2 * QB - offb],
                    ps[:, offa : 2 * QB - offb],
                    AF.Exp,
                    scale=SCALE,
                )
                # causal masking: only the 128-col crossing region of
                # diagonal k-tiles, multiplied in place on the vector engine
                for i, col0 in ((ia, offa), (ib, QB)):
                    if i >= 4 * qb:
                        cl = slice(col0, col0 + P)
                        nc.vector.tensor_mul(p_sb[:, cl], p_sb[:, cl], mask_sb[:])
                ptiles[(qb, m)] = p_sb

            def av_pair(qb, m):
                ia, ib = 2 * m, 2 * m + 1
                offa = max(0, P * ia - QB * qb)
                offb = max(0, P * ib - QB * qb)
                last = 4 * qb + 3
                p_sb = ptiles.pop((qb, m))
                po = pos[qb]
                nc.tensor.matmul(
                    po[:, offa:],
                    v_sb[:, ia, :],
                    p_sb[:, offa:QB],
                    start=(ia == 0),
                    stop=False,
                    skip_group_check=True,
                )
                nc.tensor.matmul(
                    po[:, offb:],
                    v_sb[:, ib, :],
                    p_sb[:, QB : 2 * QB - offb],
                    start=False,
                    stop=(ib == last),
                    skip_group_check=True,
                )

            def start_block(qb):
                pos[qb] = ps_o.tile([H + 1, QB], F32, tag="po", name=f"po_{qb}")

            def finish_block(qb):
                po = pos.pop(qb)
                cols = slice(qb * QB, (qb + 1) * QB)
                nc.vector.tensor_copy(out_sb[:, cols], po[:])
                nc.sync.dma_start(out_d[:, cols], out_sb[:, cols])

            # ---------------- schedule ----------------
            # qb-major; exp of pair m overlaps scores of m+1/m+2 and AV
            # trails; projections fill PE slack as their idx block lands
            proj_qk(0)
            s_pair(0, 0)
            s_pair(0, 1)
            proj_qk(1)
            s_pair(1, 0)
            s_pair(1, 1)
            proj_v_pair(0)          # v blocks 0+1 (k-tiles 0..7)
            transp_v(0, 0)
            transp_v(0, 4)
            start_block(0)
            av_pair(0, 0)
            av_pair(0, 1)
            finish_block(0)
            s_pair(1, 2)
            s_pair(1, 3)
            proj_qk(2)
            start_block(1)
            av_pair(1, 0)
            av_pair(1, 1)
            s_pair(2, 0)
            s_pair(2, 1)
            av_pair(1, 2)
            av_pair(1, 3)
            finish_block(1)
            s_pair(2, 2)
            s_pair(2, 3)
            proj_qk(3)
            proj_v_pair(2)          # v blocks 2+3 (k-tiles 8..15)
            s_pair(2, 4)
            s_pair(2, 5)
            transp_v(2, 0)
            transp_v(2, 4)
            start_block(2)
            av_pair(2, 0)
            av_pair(2, 1)
            av_pair(2, 2)
            s_pair(3, 0)
            s_pair(3, 1)
            av_pair(2, 3)
            av_pair(2, 4)
            s_pair(3, 2)
            s_pair(3, 3)
            av_pair(2, 5)
            finish_block(2)
            start_block(3)
            av_pair(3, 0)
            s_pair(3, 4)
            s_pair(3, 5)
            av_pair(3, 1)
            av_pair(3, 2)
            s_pair(3, 6)
            s_pair(3, 7)
            av_pair(3, 3)
            av_pair(3, 4)
            av_pair(3, 5)
            av_pair(3, 6)
            av_pair(3, 7)
            finish_block(3)
    nc.compile()
    return nc


_NC = None


def _get_nc():
    global _NC
    if _NC is None:
        _NC = build_bass()
    return _NC


def kernel(idx, Wk, Wq, Wv):
    global LAST_RESULT
    idx = np.asarray(idx, dtype=np.float32)
    Wk = np.asarray(Wk, dtype=np.float32)
    Wq = np.asarray(Wq, dtype=np.float32)
    Wv = np.asarray(Wv, dtype=np.float32)

    # weight swizzle: w*[p, t*M + m] = W[t*128 + p, m]
    wqk = np.concatenate([Wq, Wk], axis=1)  # [D, 128]
    wqk = np.ascontiguousarray(
        wqk.reshape(DT, P, 2 * H).transpose(1, 0, 2).reshape(P, DT * 2 * H)
    ).astype(ml_dtypes.bfloat16)
    wv = np.ascontiguousarray(
        Wv.reshape(DT, P, H).transpose(1, 0, 2).reshape(P, DT * H)
    ).astype(ml_dtypes.bfloat16)
    in_maps = []
    for i in range(B):
        idxT = np.ascontiguousarray(idx[i].T).astype(ml_dtypes.bfloat16)  # [D, S]
        # block-major swizzle: [b, p, t*QB + s] = idxT[t*P + p, b*QB + s]
        blk = np.ascontiguousarray(
            idxT.reshape(DT, P, NB, QB).transpose(2, 1, 0, 3).reshape(NB, P, DT * QB)
        )
        in_maps.append({"idxT": blk, "wqk": wqk, "wv": wv})

    res = run_bass_kernel_spmd(_get_nc(), in_maps, core_ids=list(range(B)), trace=TRACE)
    LAST_RESULT = res

    out = np.empty((B, S, H), dtype=np.float32)
    for i in range(B):
        o = np.asarray(res.results[i]["out"], dtype=np.float32)  # [65, S]
        out[i] = (o[0:H] / o[H : H + 1]).T
    return out


if __name__ == "__main__":
    rng = np.random.default_rng(0)
    idx = rng.standard_normal((B, S, D), dtype=np.float32)
    Wk = rng.standard_normal((D, H), dtype=np.float32) / np.sqrt(D)
    Wq = rng.standard_normal((D, H), dtype=np.float32) / np.sqrt(D)
    Wv = rng.standard_normal((D, H), dtype=np.float32) / np.sqrt(D)
    o = kernel(idx=idx, Wk=Wk, Wq=Wq, Wv=Wv)
    print(o.shape, o.dtype, np.abs(o).mean())


# revision 24
# speedup vs baseline: 1.0307x; 1.0307x over previous
"""Single-head causal attention (B=8, S=2048, D=1024, H=64).

Data-parallel over batch: each of the 8 NeuronCores computes one batch
element's full attention head.  Per core:

  qT/kT = (Wq|Wk)^T @ idx^T          -> [128, S]  (rows 0-63 qT, 64-127 kT)
  vT    = Wv^T @ idx^T               -> col-packed pairs of q-blocks
  v     = transpose(vT)              -> [S/128 x 128, 64] + ones column
  sT[k,q] = kT_tile^T @ qT           -> row-packed pairs (k on partitions)
  p = exp(sT / sqrt(D)) * causal     (no max subtraction: |s| <= ~2)
  oT[65, q] += v_aug[k]^T @ p        -> rows 0-63 out, row 64 = sum(exp) = Z

The kernel stores oT = [65, S] f32 (unnormalized + Z row); the HOST does
out[q, h] = oT[h, q] / oT[64, q] -- no on-chip transpose or divide.

Schedule: the framework preamble releases the engines at ~7us; from
there everything is input-DMA-gated (4.3MB bf16 at ~360-500 GB/s), so
idx blocks stream on sync+scalar HWDGE rings in block order while
weights ride the gpsimd SWDGE ring.  Attention is a qb-major pipeline:
scores for pair m+1/m+2 overlap exp of pair m on the ACT engine
(exp ~20us serial is the co-pole with the PE at ~23us), AV matmuls
trail exp, projections fill PE slack as their block lands.
"""

import sys

for _p in ("/opt/trn_rl_repo",):
    if _p not in sys.path:
        sys.path.insert(0, _p)

import numpy as np
import ml_dtypes

import concourse.bacc as bacc
import concourse.bass as bass
import concourse.mybir as mybir
from concourse import masks, tile
from concourse.bass_utils import run_bass_kernel_spmd

B, S, D, H = 8, 2048, 1024, 64
P = 128
QB = 512            # q-block width (one PSUM bank of f32)
NB = S // QB        # 4 q-blocks
KT = S // P         # 16 k-tiles
DT = D // P         # 8 d-tiles
SCALE = float(D) ** -0.5  # 1/32, exact in bf16/f32

BF16 = mybir.dt.bfloat16
F32 = mybir.dt.float32
AF = mybir.ActivationFunctionType

TRACE = False
LAST_RESULT = None


def enable_trace():
    """Register the NTFF profile hook that the agent image's antenv lacks,
    and neuter the artifact upload (no bucket in this container)."""
    global TRACE
    import types

    import antenv
    import concourse.bass_utils as bu

    if "antenv.axon_hooks" not in sys.modules:
        mod = types.ModuleType("antenv.axon_hooks")
        mod._hook = None
        mod.set_axon_ntff_profile_hook = lambda h: setattr(mod, "_hook", h)
        mod.get_axon_ntff_profile_hook = lambda: mod._hook
        sys.modules["antenv.axon_hooks"] = mod
        antenv.axon_hooks = mod
    from trn_agent_boot.trn_boot import _ntff_profile_via_ctypes

    sys.modules["antenv.axon_hooks"].set_axon_ntff_profile_hook(
        _ntff_profile_via_ctypes("/opt/axon/libaxon_pjrt.so")
    )
    bu.upload_artifacts = lambda tmpdir: tmpdir
    TRACE = True


def build_bass():
    nc = bacc.Bacc("TRN2", target_bir_lowering=False, debug=False, num_devices=B)
    # block-major swizzle: idxT_d[b, p, t*QB + s] = idx[b*QB + s, t*P + p]
    idxT_d = nc.declare_dram_parameter("idxT", [NB, P, DT * QB], BF16, isOutput=False)
    # host pre-swizzled: wqk_d[p, t*2H + m] = concat(Wq,Wk)[t*P + p, m]
    wqk_d = nc.declare_dram_parameter("wqk", [P, DT * 2 * H], BF16, isOutput=False)
    wv_d = nc.declare_dram_parameter("wv", [P, DT * H], BF16, isOutput=False)
    # oT[h, q]: rows 0-63 unnormalized out, row 64 = Z (host divides)
    out_d = nc.declare_dram_parameter("out", [H + 1, S], F32, isOutput=True)

    with tile.TileContext(nc) as tc:
        with (
            tc.tile_pool(name="consts", bufs=1) as consts,
            tc.tile_pool(name="data", bufs=1) as data,
            tc.tile_pool(name="pp", bufs=8) as pp,
            tc.tile_pool(name="ps_mm", bufs=2, space="PSUM") as ps_mm,
            tc.tile_pool(name="ps_pj", bufs=2, space="PSUM") as ps_pj,
            tc.tile_pool(name="ps_o", bufs=2, space="PSUM") as ps_o,
        ):
            # ---------------- input DMAs first ----------------
            # idx blocks in need-order: 0a/0b/1 on the sync HWDGE ring,
            # 2/3 on the scalar HWDGE ring, weights on the gpsimd SWDGE
            idxT_sb = data.tile([P, NB, DT, QB], BF16)

            def load_idx(eng, b, splits=1):
                src = idxT_d[b].rearrange("p (t s) -> p t s", t=DT)
                step = DT // splits
                for u in range(splits):
                    ts = slice(u * step, (u + 1) * step)
                    eng.dma_start(idxT_sb[:, b, ts, :], src[:, ts, :])

            # weights ride the sync HWDGE ring (fast, lands ~9.5us); all idx
            # blocks stream in block order on the scalar HWDGE ring, which
            # alone sustains ~360 B/ns; the sync ring then carries the
            # output stores.  The dummy exp (ACT table load, ~1.3us queue
            # time) slots between idx0's and idx1's issues so the table is
            # resident well before the first real exp without delaying idx0.
            src0 = idxT_d[0].rearrange("p (t s) -> p t s", t=DT)
            nc.sync.dma_start(idxT_sb[:, 0, 0:4, :], src0[:, 0:4, :])
            wqk_sb = consts.tile([P, DT, 2 * H], BF16)
            nc.scalar.dma_start(wqk_sb[:], wqk_d.rearrange("p (t m) -> p t m", t=DT))
            nc.scalar.dma_start(idxT_sb[:, 0, 4:8, :], src0[:, 4:8, :])
            wv_sb = consts.tile([P, DT, H], BF16)
            nc.sync.dma_start(wv_sb[:], wv_d.rearrange("p (t m) -> p t m", t=DT))
            dum = consts.tile([1, 16], F32)
            nc.scalar.activation(dum[:, 0:8], dum[:, 8:16], AF.Exp)
            load_idx(nc.scalar, 1)
            load_idx(nc.scalar, 2)
            load_idx(nc.scalar, 3, splits=2)

            # ---------------- PE warmup ----------------
            junk = consts.tile([P, QB], BF16)
            nc.vector.memset(junk[:], 0.0)
            wps = ps_pj.tile([P, QB], F32, tag="pj")
            for _ in range(6):
                nc.tensor.matmul(wps[:], junk[:, 0:P], junk[:], start=True, stop=True)

            # ---------------- constants ----------------
            ident = consts.tile([P, P], BF16)
            masks.make_identity(nc, ident[:])
            # mask[k, q] = 1.0 where q >= k (upper triangular incl diagonal)
            mask_sb = consts.tile([P, P], BF16)
            masks.make_upper_triangular(nc, mask_sb[:], val=1.0, diag=True)
            # half-swap permutation: ident_sw[p, i] = 1 iff |p - i| == 64;
            # matmul with it as lhsT moves partitions 64-127 <-> 0-63
            ident_sw = consts.tile([P, P], BF16)
            nc.gpsimd.memset(ident_sw[:], 0.0)
            for sw_base in (H, -H):
                nc.gpsimd.affine_select(
                    out=ident_sw[:],
                    in_=ident_sw[:],
                    compare_op=mybir.AluOpType.not_equal,
                    fill=1.0,
                    base=sw_base,
                    pattern=[[-1, P]],
                    channel_multiplier=1,
                )
            # identity on partitions 64-127 (for transposing hi-half vT)
            identh = consts.tile([P, H], BF16)
            nc.gpsimd.memset(identh[:], 0.0)
            nc.gpsimd.affine_select(
                out=identh[:],
                in_=identh[:],
                compare_op=mybir.AluOpType.not_equal,
                fill=1.0,
                base=-H,
                pattern=[[-1, H]],
                channel_multiplier=1,
            )

            # ---------------- working tiles ----------------
            qkT_sb = data.tile([P, S], BF16)   # rows 0-63 qT, rows 64-127 kT
            qkT2_sb = data.tile([P, S], BF16)  # swapped: rows 0-63 kT, 64-127 qT
            vT_sb = data.tile([P, S // 2], BF16)  # even blocks rows 0-63, odd 64-127
            v_sb = data.tile([P, KT, H + 1], BF16)  # [k, 64 v | 1.0]
            out_sb = data.tile([H + 1, S], F32)

            # ones column of v_aug (flash-style Z accumulator row)
            nc.vector.memset(v_sb[:, :, H : H + 1].rearrange("p t o -> p (t o)"), 1.0)

            def proj_qk(b, chunks=1):
                cols = slice(b * QB, (b + 1) * QB)
                ps = ps_pj.tile([P, QB], F32, tag="pj", name=f"pqk{b}")
                for t in range(DT):
                    nc.tensor.matmul(
                        ps[:],
                        wqk_sb[:, t, :],
                        idxT_sb[:, b, t, :],
                        start=(t == 0),
                        stop=(t == DT - 1),
                    )
                nc.vector.tensor_copy(qkT_sb[:, cols], ps[:])
                # scores row-packing needs kT on partitions 0-63 (pair slot A)
                # and qT on partitions 64-127 (pair slot B): swap the halves
                # on the PE via the permutation identity (no DMA latency)
                ps2 = ps_pj.tile([P, QB], F32, tag="pj", name=f"sw{b}")
                nc.tensor.matmul(
                    ps2[:], ident_sw[:], qkT_sb[:, cols], start=True, stop=True
                )
                nc.vector.tensor_copy(qkT2_sb[:, cols], ps2[:])

            def proj_v_pair(b):
                # col-packed pair: vT(b) -> psum parts 0-63, vT(b+1) -> 64-127
                hcols = slice(b * QB // 2, (b + 2) * QB // 2)
                ps = ps_pj.tile([P, QB], F32, tag="pj", name=f"pv{b}")
                for t in range(DT):
                    nc.tensor.matmul(
                        ps[0:H, :],
                        wv_sb[:, t, :],
                        idxT_sb[:, b, t, :],
                        start=(t == 0),
                        stop=(t == DT - 1),
                    )
                    nc.tensor.matmul(
                        ps[H:P, :],
                        wv_sb[:, t, :],
                        idxT_sb[:, b + 1, t, :],
                        start=(t == 0),
                        stop=(t == DT - 1),
                        tile_position=(0, H),
                    )
                nc.vector.tensor_copy(vT_sb[:, hcols], ps[:])

            def transp_v(b, g):
                # transpose 4 k-tiles of vT (one PSUM bank, single DVE evac)
                j0 = 4 * b + g
                pst = ps_o.tile([P, 4, H], BF16, tag="po", name=f"vt_{b}_{g}")
                for u in range(4):
                    j = j0 + u
                    jj = (j - 4 * b) % 4 + 2 * b  # column tile within vT_sb
                    hi = j >= 4 * b + 4
                    src = (
                        vT_sb[H:P, jj * P : (jj + 1) * P]
                        if hi
                        else vT_sb[0:H, jj * P : (jj + 1) * P]
                    )
                    nc.tensor.matmul(
                        pst[:, u, :],
                        src,
                        identh[H:P, :] if hi else ident[:H, :H],
                        is_transpose=True,
                        start=(u == 0),
                        stop=(u == 3),
                        skip_group_check=True,
                    )
                nc.vector.tensor_copy(v_sb[:, j0 : j0 + 4, 0:H], pst[:])

            # --------- attention: S-pair -> exp -> masked AV pair ---------
            pos = {}
            ptiles = {}

            def s_pair(qb, m):
                """Scores for k-tiles (2m, 2m+1) x q-block qb, then exp.
                B half is packed at column base QB (not QB+offb) so the
                single exp activation covers only valid columns."""
                ia, ib = 2 * m, 2 * m + 1
                offa = max(0, P * ia - QB * qb)
                offb = max(0, P * ib - QB * qb)
                ps = ps_mm.tile([P, 2 * QB], F32, tag="mm", name=f"s{qb}_{m}")
                nc.tensor.matmul(
                    ps[:, offa:QB],
                    qkT2_sb[0:H, ia * P : (ia + 1) * P],
                    qkT_sb[0:H, qb * QB + offa : (qb + 1) * QB],
                    start=True,
                    stop=True,
                )
                nc.tensor.matmul(
                    ps[:, QB : 2 * QB - offb],
                    qkT_sb[H:P, ib * P : (ib + 1) * P],
                    qkT2_sb[H:P, qb * QB + offb : (qb + 1) * QB],
                    start=True,
                    stop=True,
                )
                p_sb = pp.tile([P, 2 * QB], BF16, tag="p", name=f"p{qb}_{m}")
                nc.scalar.activation(
                    p_sb[:, offa : 2 * QB - offb],
                    ps[:, offa : 2 * QB - offb],
                    AF.Exp,
                    scale=SCALE,
                )
                ptiles[(qb, m)] = p_sb

            def av_pair(qb, m):
                ia, ib = 2 * m, 2 * m + 1
                offa = max(0, P * ia - QB * qb)
                offb = max(0, P * ib - QB * qb)
                last = 4 * qb + 3
                p_sb = ptiles.pop((qb, m))
                po = pos[qb]
                # causal masking: only the 128-col crossing region of
                # diagonal k-tiles; emitted at AV time so the DVE queue
                # never holds proj evacuations behind exp-gated masks
                for i, col0 in ((ia, offa), (ib, QB)):
                    if i >= 4 * qb:
                        cl = slice(col0, col0 + P)
                        nc.vector.tensor_mul(p_sb[:, cl], p_sb[:, cl], mask_sb[:])
                nc.tensor.matmul(
                    po[:, offa:],
                    v_sb[:, ia, :],
                    p_sb[:, offa:QB],
                    start=(ia == 0),
                    stop=False,
                    skip_group_check=True,
                )
                nc.tensor.matmul(
                    po[:, offb:],
                    v_sb[:, ib, :],
                    p_sb[:, QB : 2 * QB - offb],
                    start=False,
                    stop=(ib == last),
                    skip_group_check=True,
                )

            def start_block(qb):
                pos[qb] = ps_o.tile([H + 1, QB], F32, tag="po", name=f"po_{qb}")

            def finish_block(qb):
                po = pos.pop(qb)
                cols = slice(qb * QB, (qb + 1) * QB)
                nc.vector.tensor_copy(out_sb[:, cols], po[:])
                nc.sync.dma_start(out_d[:, cols], out_sb[:, cols])

            # ---------------- schedule ----------------
            # projections march as their idx block lands, each followed by
            # its PE half-swap; S-pairs keep the ACT engine saturated; AV
            # pairs + v-plumbing fill remaining PE slack
            proj_qk(0)
            s_pair(0, 0)
            s_pair(0, 1)
            proj_qk(1)
            s_pair(1, 0)
            s_pair(1, 1)
            proj_v_pair(0)          # v blocks 0+1 (k-tiles 0..7)
            transp_v(0, 0)
            transp_v(0, 4)
            s_pair(1, 2)
            s_pair(1, 3)
            proj_qk(2)
            start_block(0)
            av_pair(0, 0)
            av_pair(0, 1)
            finish_block(0)
            s_pair(2, 0)
            s_pair(2, 1)
            start_block(1)
            av_pair(1, 0)
            av_pair(1, 1)
            s_pair(2, 2)
            s_pair(2, 3)
            proj_qk(3)
            av_pair(1, 2)
            av_pair(1, 3)
            finish_block(1)
            proj_v_pair(2)          # v blocks 2+3 (k-tiles 8..15)
            transp_v(2, 0)
            transp_v(2, 4)
            s_pair(2, 4)
            s_pair(2, 5)
            s_pair(3, 0)
            s_pair(3, 1)
            start_block(2)
            av_pair(2, 0)
            av_pair(2, 1)
            av_pair(2, 2)
            s_pair(3, 2)
            s_pair(3, 3)
            av_pair(2, 3)
            av_pair(2, 4)
            av_pair(2, 5)
            finish_block(2)
            s_pair(3, 4)
            s_pair(3, 5)
            start_block(3)
            av_pair(3, 0)
            av_pair(3, 1)
            s_pair(3, 6)
            s_pair(3, 7)
            av_pair(3, 2)
            av_pair(3, 3)
            av_pair(3, 4)
            av_pair(3, 5)
            av_pair(3, 6)
            av_pair(3, 7)
            finish_block(3)
    nc.compile()
    return nc


_NC = None


def _get_nc():
    global _NC
    if _NC is None:
        _NC = build_bass()
    return _NC


def kernel(idx, Wk, Wq, Wv):
    global LAST_RESULT
    idx = np.asarray(idx, dtype=np.float32)
    Wk = np.asarray(Wk, dtype=np.float32)
    Wq = np.asarray(Wq, dtype=np.float32)
    Wv = np.asarray(Wv, dtype=np.float32)

    # weight swizzle: w*[p, t*M + m] = W[t*128 + p, m]
    wqk = np.concatenate([Wq, Wk], axis=1)  # [D, 128]
    wqk = np.ascontiguousarray(
        wqk.reshape(DT, P, 2 * H).transpose(1, 0, 2).reshape(P, DT * 2 * H)
    ).astype(ml_dtypes.bfloat16)
    wv = np.ascontiguousarray(
        Wv.reshape(DT, P, H).transpose(1, 0, 2).reshape(P, DT * H)
    ).astype(ml_dtypes.bfloat16)
    in_maps = []
    for i in range(B):
        idxT = np.ascontiguousarray(idx[i].T).astype(ml_dtypes.bfloat16)  # [D, S]
        # block-major swizzle: [b, p, t*QB + s] = idxT[t*P + p, b*QB + s]
        blk = np.ascontiguousarray(
            idxT.reshape(DT, P, NB, QB).transpose(2, 1, 0, 3).reshape(NB, P, DT * QB)
        )
        in_maps.append({"idxT": blk, "wqk": wqk, "wv": wv})

    res = run_bass_kernel_spmd(_get_nc(), in_maps, core_ids=list(range(B)), trace=TRACE)
    LAST_RESULT = res

    out = np.empty((B, S, H), dtype=np.float32)
    for i in range(B):
        o = np.asarray(res.results[i]["out"], dtype=np.float32)  # [65, S]
        out[i] = (o[0:H] / o[H : H + 1]).T
    return out


if __name__ == "__main__":
    rng = np.random.default_rng(0)
    idx = rng.standard_normal((B, S, D), dtype=np.float32)
    Wk = rng.standard_normal((D, H), dtype=np.float32) / np.sqrt(D)
    Wq = rng.standard_normal((D, H), dtype=np.float32) / np.sqrt(D)
    Wv = rng.standard_normal((D, H), dtype=np.float32) / np.sqrt(D)
    o = kernel(idx=idx, Wk=Wk, Wq=Wq, Wv=Wv)
    print(o.shape, o.dtype, np.abs(o).mean())


# revision 25
# speedup vs baseline: 1.0737x; 1.0418x over previous
"""Single-head causal attention (B=8, S=2048, D=1024, H=64).

Data-parallel over batch: each of the 8 NeuronCores computes one batch
element's full attention head.  Per core:

  qT/kT = (Wq|Wk)^T @ idx^T          -> [128, S]  (rows 0-63 qT, 64-127 kT)
  vT    = Wv^T @ idx^T               -> col-packed pairs of q-blocks
  v     = transpose(vT)              -> [S/128 x 128, 64] + ones column
  sT[k,q] = kT_tile^T @ qT           -> row-packed pairs (k on partitions)
  p = exp(sT / sqrt(D)) * causal     (no max subtraction: |s| <= ~2)
  oT[65, q] += v_aug[k]^T @ p        -> rows 0-63 out, row 64 = sum(exp) = Z

The kernel stores oT = [65, S] f32 (unnormalized + Z row); the HOST does
out[q, h] = oT[h, q] / oT[64, q] -- no on-chip transpose or divide.

Schedule: the framework preamble releases the engines at ~7us; from
there everything is input-DMA-gated (4.3MB bf16 at ~360-500 GB/s), so
idx blocks stream on sync+scalar HWDGE rings in block order while
weights ride the gpsimd SWDGE ring.  Attention is a qb-major pipeline:
scores for pair m+1/m+2 overlap exp of pair m on the ACT engine
(exp ~20us serial is the co-pole with the PE at ~23us), AV matmuls
trail exp, projections fill PE slack as their block lands.
"""

import sys

for _p in ("/opt/trn_rl_repo",):
    if _p not in sys.path:
        sys.path.insert(0, _p)

import numpy as np
import ml_dtypes

import concourse.bacc as bacc
import concourse.bass as bass
import concourse.mybir as mybir
from concourse import masks, tile
from concourse.bass_utils import run_bass_kernel_spmd

B, S, D, H = 8, 2048, 1024, 64
P = 128
QB = 512            # q-block width (one PSUM bank of f32)
NB = S // QB        # 4 q-blocks
KT = S // P         # 16 k-tiles
DT = D // P         # 8 d-tiles
SCALE = float(D) ** -0.5  # 1/32, exact in bf16/f32

BF16 = mybir.dt.bfloat16
F32 = mybir.dt.float32
AF = mybir.ActivationFunctionType

TRACE = False
LAST_RESULT = None


def enable_trace():
    """Register the NTFF profile hook that the agent image's antenv lacks,
    and neuter the artifact upload (no bucket in this container)."""
    global TRACE
    import types

    import antenv
    import concourse.bass_utils as bu

    if "antenv.axon_hooks" not in sys.modules:
        mod = types.ModuleType("antenv.axon_hooks")
        mod._hook = None
        mod.set_axon_ntff_profile_hook = lambda h: setattr(mod, "_hook", h)
        mod.get_axon_ntff_profile_hook = lambda: mod._hook
        sys.modules["antenv.axon_hooks"] = mod
        antenv.axon_hooks = mod
    from trn_agent_boot.trn_boot import _ntff_profile_via_ctypes

    sys.modules["antenv.axon_hooks"].set_axon_ntff_profile_hook(
        _ntff_profile_via_ctypes("/opt/axon/libaxon_pjrt.so")
    )
    bu.upload_artifacts = lambda tmpdir: tmpdir
    TRACE = True


def build_bass():
    nc = bacc.Bacc("TRN2", target_bir_lowering=False, debug=False, num_devices=B)
    # block-major swizzle: idxT_d[b, p, t*QB + s] = idx[b*QB + s, t*P + p]
    idxT_d = nc.declare_dram_parameter("idxT", [NB, P, DT * QB], BF16, isOutput=False)
    # host pre-swizzled: wqk_d[p, t*2H + m] = concat(Wq,Wk)[t*P + p, m]
    wqk_d = nc.declare_dram_parameter("wqk", [P, DT * 2 * H], BF16, isOutput=False)
    wv_d = nc.declare_dram_parameter("wv", [P, DT * H], BF16, isOutput=False)
    # oT[h, q]: rows 0-63 unnormalized out, row 64 = Z (host divides)
    out_d = nc.declare_dram_parameter("out", [H + 1, S], F32, isOutput=True)

    with tile.TileContext(nc) as tc:
        with (
            tc.tile_pool(name="consts", bufs=1) as consts,
            tc.tile_pool(name="data", bufs=1) as data,
            tc.tile_pool(name="pp", bufs=8) as pp,
            tc.tile_pool(name="ps_mm", bufs=3, space="PSUM") as ps_mm,
            tc.tile_pool(name="ps_pj", bufs=1, space="PSUM") as ps_pj,
            tc.tile_pool(name="ps_o", bufs=1, space="PSUM") as ps_o,
        ):
            # ---------------- input DMAs first ----------------
            # idx blocks in need-order: 0a/0b/1 on the sync HWDGE ring,
            # 2/3 on the scalar HWDGE ring, weights on the gpsimd SWDGE
            idxT_sb = data.tile([P, NB, DT, QB], BF16)

            def load_idx(eng, b, splits=1):
                src = idxT_d[b].rearrange("p (t s) -> p t s", t=DT)
                step = DT // splits
                for u in range(splits):
                    ts = slice(u * step, (u + 1) * step)
                    eng.dma_start(idxT_sb[:, b, ts, :], src[:, ts, :])

            # weights ride the sync HWDGE ring (fast, lands ~9.5us); all idx
            # blocks stream in block order on the scalar HWDGE ring, which
            # alone sustains ~360 B/ns; the sync ring then carries the
            # output stores.  The dummy exp (ACT table load, ~1.3us queue
            # time) slots between idx0's and idx1's issues so the table is
            # resident well before the first real exp without delaying idx0.
            wqk_sb = consts.tile([P, DT, 2 * H], BF16)
            nc.sync.dma_start(wqk_sb[:], wqk_d.rearrange("p (t m) -> p t m", t=DT))
            wv_sb = consts.tile([P, DT, H], BF16)
            nc.sync.dma_start(wv_sb[:], wv_d.rearrange("p (t m) -> p t m", t=DT))
            dum = consts.tile([1, 16], F32)
            load_idx(nc.scalar, 0, splits=2)
            nc.scalar.activation(dum[:, 0:8], dum[:, 8:16], AF.Exp)
            load_idx(nc.scalar, 1)
            load_idx(nc.scalar, 2)
            load_idx(nc.scalar, 3, splits=2)

            # ---------------- PE warmup ----------------
            junk = consts.tile([P, QB], BF16)
            nc.vector.memset(junk[:], 0.0)
            wps = ps_pj.tile([P, QB], F32, tag="pj")
            for _ in range(10):
                nc.tensor.matmul(wps[:], junk[:, 0:P], junk[:], start=True, stop=True)

            # ---------------- constants ----------------
            ident = consts.tile([P, P], BF16)
            masks.make_identity(nc, ident[:])
            # mask[k, q] = 1.0 where q >= k (upper triangular incl diagonal)
            mask_sb = consts.tile([P, P], BF16)
            masks.make_upper_triangular(nc, mask_sb[:], val=1.0, diag=True)
            # half-swap permutation: ident_sw[p, i] = 1 iff |p - i| == 64;
            # matmul with it as lhsT moves partitions 64-127 <-> 0-63
            ident_sw = consts.tile([P, P], BF16)
            nc.gpsimd.memset(ident_sw[:], 0.0)
            for sw_base in (H, -H):
                nc.gpsimd.affine_select(
                    out=ident_sw[:],
                    in_=ident_sw[:],
                    compare_op=mybir.AluOpType.not_equal,
                    fill=1.0,
                    base=sw_base,
                    pattern=[[-1, P]],
                    channel_multiplier=1,
                )
            # identity on partitions 64-127 (for transposing hi-half vT)
            identh = consts.tile([P, H], BF16)
            nc.gpsimd.memset(identh[:], 0.0)
            nc.gpsimd.affine_select(
                out=identh[:],
                in_=identh[:],
                compare_op=mybir.AluOpType.not_equal,
                fill=1.0,
                base=-H,
                pattern=[[-1, H]],
                channel_multiplier=1,
            )

            # ---------------- working tiles ----------------
            qkT_sb = data.tile([P, S], BF16)   # rows 0-63 qT, rows 64-127 kT
            qkT2_sb = data.tile([P, S], BF16)  # swapped: rows 0-63 kT, 64-127 qT
            vT_sb = data.tile([P, S // 2], BF16)  # even blocks rows 0-63, odd 64-127
            v_sb = data.tile([P, KT, H + 1], BF16)  # [k, 64 v | 1.0]
            out_sb = data.tile([H + 1, S], F32)

            # ones column of v_aug (flash-style Z accumulator row)
            nc.vector.memset(v_sb[:, :, H : H + 1].rearrange("p t o -> p (t o)"), 1.0)

            def proj_qk(b, chunks=1):
                cols = slice(b * QB, (b + 1) * QB)
                ps = ps_pj.tile([P, QB], F32, tag="pj", name=f"pqk{b}")
                for t in range(DT):
                    nc.tensor.matmul(
                        ps[:],
                        wqk_sb[:, t, :],
                        idxT_sb[:, b, t, :],
                        start=(t == 0),
                        stop=(t == DT - 1),
                    )
                nc.vector.tensor_copy(qkT_sb[:, cols], ps[:])
                # scores row-packing needs kT on partitions 0-63 (pair slot A)
                # and qT on partitions 64-127 (pair slot B): swap the halves
                # on the PE via the permutation identity (no DMA latency)
                ps2 = ps_pj.tile([P, QB], F32, tag="pj", name=f"sw{b}")
                nc.tensor.matmul(
                    ps2[:], ident_sw[:], qkT_sb[:, cols], start=True, stop=True
                )
                nc.vector.tensor_copy(qkT2_sb[:, cols], ps2[:])

            def proj_v_pair(b):
                # col-packed pair: vT(b) -> psum parts 0-63, vT(b+1) -> 64-127
                hcols = slice(b * QB // 2, (b + 2) * QB // 2)
                ps = ps_pj.tile([P, QB], F32, tag="pj", name=f"pv{b}")
                for t in range(DT):
                    nc.tensor.matmul(
                        ps[0:H, :],
                        wv_sb[:, t, :],
                        idxT_sb[:, b, t, :],
                        start=(t == 0),
                        stop=(t == DT - 1),
                    )
                    nc.tensor.matmul(
                        ps[H:P, :],
                        wv_sb[:, t, :],
                        idxT_sb[:, b + 1, t, :],
                        start=(t == 0),
                        stop=(t == DT - 1),
                        tile_position=(0, H),
                    )
                nc.vector.tensor_copy(vT_sb[:, hcols], ps[:])

            def transp_v(b, g):
                # transpose 4 k-tiles of vT (one PSUM bank, single DVE evac)
                j0 = 4 * b + g
                pst = ps_pj.tile([P, 4, H], BF16, tag="pj", name=f"vt_{b}_{g}")
                for u in range(4):
                    j = j0 + u
                    jj = (j - 4 * b) % 4 + 2 * b  # column tile within vT_sb
                    hi = j >= 4 * b + 4
                    src = (
                        vT_sb[H:P, jj * P : (jj + 1) * P]
                        if hi
                        else vT_sb[0:H, jj * P : (jj + 1) * P]
                    )
                    nc.tensor.matmul(
                        pst[:, u, :],
                        src,
                        identh[H:P, :] if hi else ident[:H, :H],
                        is_transpose=True,
                        start=(u == 0),
                        stop=(u == 3),
                        skip_group_check=True,
                    )
                nc.vector.tensor_copy(v_sb[:, j0 : j0 + 4, 0:H], pst[:])

            # --------- attention: S-pair -> exp -> masked AV pair ---------
            pos = {}
            ptiles = {}

            def s_pair(qb, m):
                """Scores for k-tiles (2m, 2m+1) x q-block qb, then exp.
                B half is packed at column base QB (not QB+offb) so the
                single exp activation covers only valid columns."""
                ia, ib = 2 * m, 2 * m + 1
                offa = max(0, P * ia - QB * qb)
                offb = max(0, P * ib - QB * qb)
                ps = ps_mm.tile([P, 2 * QB], F32, tag="mm", name=f"s{qb}_{m}")
                nc.tensor.matmul(
                    ps[:, offa:QB],
                    qkT2_sb[0:H, ia * P : (ia + 1) * P],
                    qkT_sb[0:H, qb * QB + offa : (qb + 1) * QB],
                    start=True,
                    stop=True,
                )
                nc.tensor.matmul(
                    ps[:, QB : 2 * QB - offb],
                    qkT_sb[H:P, ib * P : (ib + 1) * P],
                    qkT2_sb[H:P, qb * QB + offb : (qb + 1) * QB],
                    start=True,
                    stop=True,
                )
                p_sb = pp.tile([P, 2 * QB], BF16, tag="p", name=f"p{qb}_{m}")
                nc.scalar.activation(
                    p_sb[:, offa : 2 * QB - offb],
                    ps[:, offa : 2 * QB - offb],
                    AF.Exp,
                    scale=SCALE,
                )
                # causal masking: only the 128-col crossing region of
                # diagonal k-tiles, multiplied in place on the vector engine
                for i, col0 in ((ia, offa), (ib, QB)):
                    if i >= 4 * qb:
                        cl = slice(col0, col0 + P)
                        nc.vector.tensor_mul(p_sb[:, cl], p_sb[:, cl], mask_sb[:])
                ptiles[(qb, m)] = p_sb

            def av_pair(qb, m):
                ia, ib = 2 * m, 2 * m + 1
                offa = max(0, P * ia - QB * qb)
                offb = max(0, P * ib - QB * qb)
                last = 4 * qb + 3
                p_sb = ptiles.pop((qb, m))
                po = pos[qb]
                nc.tensor.matmul(
                    po[:, offa:],
                    v_sb[:, ia, :],
                    p_sb[:, offa:QB],
                    start=(ia == 0),
                    stop=False,
                    skip_group_check=True,
                )
                nc.tensor.matmul(
                    po[:, offb:],
                    v_sb[:, ib, :],
                    p_sb[:, QB : 2 * QB - offb],
                    start=False,
                    stop=(ib == last),
                    skip_group_check=True,
                )

            def start_block(qb):
                pos[qb] = ps_o.tile([H + 1, QB], F32, tag="po", name=f"po_{qb}")

            def finish_block(qb):
                po = pos.pop(qb)
                cols = slice(qb * QB, (qb + 1) * QB)
                nc.vector.tensor_copy(out_sb[:, cols], po[:])
                nc.sync.dma_start(out_d[:, cols], out_sb[:, cols])

            # ---------------- schedule ----------------
            # qb-major; exp of pair m overlaps scores of m+1/m+2 and AV
            # trails; projections fill PE slack as their idx block lands
            proj_qk(0)
            s_pair(0, 0)
            s_pair(0, 1)
            proj_qk(1)
            s_pair(1, 0)
            s_pair(1, 1)
            proj_v_pair(0)          # v blocks 0+1 (k-tiles 0..7)
            transp_v(0, 0)
            transp_v(0, 4)
            start_block(0)
            av_pair(0, 0)
            av_pair(0, 1)
            finish_block(0)
            s_pair(1, 2)
            s_pair(1, 3)
            proj_qk(2)
            start_block(1)
            av_pair(1, 0)
            av_pair(1, 1)
            s_pair(2, 0)
            s_pair(2, 1)
            av_pair(1, 2)
            av_pair(1, 3)
            finish_block(1)
            s_pair(2, 2)
            s_pair(2, 3)
            proj_qk(3)
            proj_v_pair(2)          # v blocks 2+3 (k-tiles 8..15)
            s_pair(2, 4)
            s_pair(2, 5)
            transp_v(2, 0)
            transp_v(2, 4)
            start_block(2)
            av_pair(2, 0)
            av_pair(2, 1)
            av_pair(2, 2)
            s_pair(3, 0)
            s_pair(3, 1)
            av_pair(2, 3)
            av_pair(2, 4)
            s_pair(3, 2)
            s_pair(3, 3)
            av_pair(2, 5)
            finish_block(2)
            start_block(3)
            av_pair(3, 0)
            s_pair(3, 4)
            s_pair(3, 5)
            av_pair(3, 1)
            av_pair(3, 2)
            s_pair(3, 6)
            s_pair(3, 7)
            av_pair(3, 3)
            av_pair(3, 4)
            av_pair(3, 5)
            av_pair(3, 6)
            av_pair(3, 7)
            finish_block(3)
    nc.compile()
    return nc


_NC = None


def _get_nc():
    global _NC
    if _NC is None:
        _NC = build_bass()
    return _NC


def kernel(idx, Wk, Wq, Wv):
    global LAST_RESULT
    idx = np.asarray(idx, dtype=np.float32)
    Wk = np.asarray(Wk, dtype=np.float32)
    Wq = np.asarray(Wq, dtype=np.float32)
    Wv = np.asarray(Wv, dtype=np.float32)

    # weight swizzle: w*[p, t*M + m] = W[t*128 + p, m]
    wqk = np.concatenate([Wq, Wk], axis=1)  # [D, 128]
    wqk = np.ascontiguousarray(
        wqk.reshape(DT, P, 2 * H).transpose(1, 0, 2).reshape(P, DT * 2 * H)
    ).astype(ml_dtypes.bfloat16)
    wv = np.ascontiguousarray(
        Wv.reshape(DT, P, H).transpose(1, 0, 2).reshape(P, DT * H)
    ).astype(ml_dtypes.bfloat16)
    in_maps = []
    for i in range(B):
        idxT = np.ascontiguousarray(idx[i].T).astype(ml_dtypes.bfloat16)  # [D, S]
        # block-major swizzle: [b, p, t*QB + s] = idxT[t*P + p, b*QB + s]
        blk = np.ascontiguousarray(
            idxT.reshape(DT, P, NB, QB).transpose(2, 1, 0, 3).reshape(NB, P, DT * QB)
        )
        in_maps.append({"idxT": blk, "wqk": wqk, "wv": wv})

    res = run_bass_kernel_spmd(_get_nc(), in_maps, core_ids=list(range(B)), trace=TRACE)
    LAST_RESULT = res

    out = np.empty((B, S, H), dtype=np.float32)
    for i in range(B):
        o = np.asarray(res.results[i]["out"], dtype=np.float32)  # [65, S]
        out[i] = (o[0:H] / o[H : H + 1]).T
    return out


if __name__ == "__main__":
    rng = np.random.default_rng(0)
    idx = rng.standard_normal((B, S, D), dtype=np.float32)
    Wk = rng.standard_normal((D, H), dtype=np.float32) / np.sqrt(D)
    Wq = rng.standard_normal((D, H), dtype=np.float32) / np.sqrt(D)
    Wv = rng.standard_normal((D, H), dtype=np.float32) / np.sqrt(D)
    o = kernel(idx=idx, Wk=Wk, Wq=Wq, Wv=Wv)
    print(o.shape, o.dtype, np.abs(o).mean())
